# revision 3
# baseline (speedup 1.0000x reference)
"""Trainium2 Bass kernel for nn_DglAggregator (GNN message passing).

Strategy (8 NeuronCores, SPMD, one uniform program, per-core data):
- Targets are partitioned across cores balanced by stage-1 edge count; each
  core owns its targets' items and ALL stage-1 edges pointing at those items,
  so no cross-core communication is needed.
- Stage 1 (item->item segment softmax + weighted sum): items laid out in
  contiguous "islot" order; windows = consecutive islot ranges (<=128 islots,
  <=1024 edges); 8 windows per batch (8192 edge ranks). Per batch the edge
  source rows are fetched with TWO bf16 dma_gathers from a per-batch
  deduplicated region (<=8192 rows, int16-addressable) of a host-relaid
  table: one normal gather ([edge, d], rhs of the ft matmul) and one
  transpose-mode gather ([d, edge], lhsT of the score matmul).
  Scores: S[e,s] = Xs_e . (h_v[dst_s] * pi) via TensorE against per-window
  dst-slot columns (no per-edge dst gather). Softmax weights: one batched
  exp (Activation engine, 4 tiles per op) + one fused scalar_tensor_tensor
  (mask = (iota==seg) * exp(S)) per tile; ft/den accumulate in PSUM via
  bf16 matmuls. Max-subtraction is skipped (|score| small, exact in f32).
- Stage 2 (item->target): same masked-matmul pattern with bf16 operands;
  per-edge ft rows come from normal + transpose dma_gathers of the stage-1
  output table (the transpose gather replaces PE transpose + copy);
  mask*weight built with fused two-op tensor_scalar.
- Numeric tables (h_v/h_p/h_t) are staged in bfloat16; all arithmetic
  (pi scaling, matmuls, softmax, tanh, means) runs on the NeuronCores with
  f32 PSUM accumulation. Host work is index math, row permutation/layout
  of input tables, and dtype staging.

kernel(**inputs) accepts the FULL unsharded inputs and returns the FULL
[N_TGT, 128] float32 output.
"""
import numpy as np
import ml_dtypes

BF16 = np.dtype(ml_dtypes.bfloat16)

P = 128          # partitions / tile edge
D = 128          # feature dim
NCORES = 8
WE1 = 1024       # stage-1 window edge capacity (8 tiles)
WS1 = 128        # stage-1 window slot capacity
WB = 8           # stage-1 windows per batch
RB = WB * WE1    # edge ranks per batch (8192)
TI2 = 50         # stage-2 tiles per window (6400 item slots)
WS2 = 128        # stage-2 window target capacity
GH = 4096        # gather granularity (half batch)
_LAST_NC = None


def _wrap_idx16(idx: np.ndarray, cap: int) -> np.ndarray:
    """[n<=cap] -> [128, cap/16] int16 (j at [j%16, j//16], replicated x8)."""
    a = np.zeros(cap, np.int64)
    a[: idx.shape[0]] = idx
    assert cap % 16 == 0
    assert a.min() >= 0 and a.max() < 32768, (a.min(), a.max())
    blk = a.reshape(cap // 16, 16).T.astype(np.int16)
    return np.tile(blk, (8, 1))


def _interleave_f32(vals: np.ndarray, cap: int, fill: float) -> np.ndarray:
    """[n] -> [128, cap/128] f32 with value of rank r at [r%128, r//128]."""
    a = np.full(cap, fill, np.float32)
    a[: vals.shape[0]] = vals
    return a.reshape(cap // P, P).T.copy()


def _pack_runs(run_sizes, max_runs, max_total):
    """Greedy pack consecutive runs into groups of whole runs, <=max_runs
    runs and <=max_total total size. Returns list of (start_run, n_runs)."""
    groups = []
    i, n = 0, len(run_sizes)
    while i < n:
        tot, j = 0, i
        while j < n and j - i < max_runs and tot + run_sizes[j] <= max_total:
            tot += run_sizes[j]
            j += 1
        assert j > i, f"run {i} of size {run_sizes[i]} exceeds {max_total}"
        groups.append((i, j - i))
        i = j
    return groups


def preprocess(h_v, h_p, h_t, int_src, int_dst, agg_dst):
    """All graph restructuring. Returns shared dims + per-core arrays."""
    NITEM = h_v.shape[0]
    NTGT = h_t.shape[0]
    int_src = int_src.astype(np.int64)
    int_dst = int_dst.astype(np.int64)
    item_tgt = agg_dst.astype(np.int64)       # item i -> target (agg_src=arange)
    h_v_bf = h_v.astype(BF16)
    h_p_bf = h_p.astype(BF16)
    h_t_bf = h_t.astype(BF16)

    # ---- target -> core, balanced by stage-1 edge load ----
    deg_int = np.bincount(int_dst, minlength=NITEM)
    t_edges = np.bincount(item_tgt, weights=deg_int.astype(np.float64),
                          minlength=NTGT)
    t_items = np.bincount(item_tgt, minlength=NTGT)
    tgt_core = np.zeros(NTGT, np.int64)
    load = np.zeros(NCORES)
    for t in np.argsort(-t_edges, kind="stable"):
        c = int(np.argmin(load))
        tgt_core[t] = c
        load[c] += t_edges[t] + 0.5 * t_items[t]
    item_core = tgt_core[item_tgt]

    cores = []
    for c in range(NCORES):
        tlist = np.where(tgt_core == c)[0]
        items = np.where(item_core == c)[0]
        items = items[np.lexsort((items, item_tgt[items]))]
        cores.append({"targets": tlist, "items": items})

    # ---- stage-2 windows (whole targets, <=WS2 targets, <=TI2*128 islots) ----
    for c in range(NCORES):
        st = cores[c]
        st["w2groups"] = _pack_runs(t_items[st["targets"]], WS2, TI2 * P)
    W2 = max(len(st["w2groups"]) for st in cores)
    NI = W2 * TI2 * P

    for c in range(NCORES):
        st = cores[c]
        tl, items = st["targets"], st["items"]
        it_item = np.full(NI, -1, np.int64)        # islot -> global item
        it_tgtloc = np.full(NI, -1.0, np.float32)  # islot -> window-local tgt
        it_tslot = np.zeros(NI, np.int64)          # islot -> global tgt slot
        twin = np.full((W2, WS2), -1, np.int64)    # window -> global targets
        ipos = 0
        for w2, (t0, ntgt) in enumerate(st["w2groups"]):
            base = w2 * TI2 * P
            off = 0
            for k in range(ntgt):
                t = tl[t0 + k]
                cnt = int(t_items[t])
                sl = slice(base + off, base + off + cnt)
                it_item[sl] = items[ipos: ipos + cnt]
                it_tgtloc[sl] = k
                it_tslot[sl] = w2 * WS2 + k
                twin[w2, k] = t
                ipos += cnt
                off += cnt
        assert ipos == len(items)
        st["it_item"] = it_item
        st["it_tgtloc"] = it_tgtloc
        st["it_tslot"] = it_tslot
        st["twin"] = twin
        islot_of = np.full(NITEM, -1, np.int64)
        real = it_item >= 0
        islot_of[it_item[real]] = np.where(real)[0]
        st["islot_of"] = islot_of

    # ---- stage-1 windows: consecutive islot ranges ----
    for c in range(NCORES):
        st = cores[c]
        emask = item_core[int_dst] == c
        es = int_src[emask]
        ed = st["islot_of"][int_dst[emask]]
        o = np.argsort(ed, kind="stable")
        st["e_src"], st["e_dst"] = es[o], ed[o]
        cnt = np.bincount(st["e_dst"], minlength=NI)
        st["w1groups"] = _pack_runs(cnt, WS1, WE1)   # (islot0, nislots)
        st["islot_cnt"] = cnt
    W1 = max(len(st["w1groups"]) for st in cores)
    W1 = ((W1 + WB - 1) // WB) * WB
    B1 = W1 // WB
    assert W1 * WS1 <= 32768, f"ft table too big for int16: W1={W1}"

    for c in range(NCORES):
        st = cores[c]
        es, ed, cnt = st["e_src"], st["e_dst"], st["islot_cnt"]
        estart = np.concatenate([[0], np.cumsum(cnt)])
        wsrc = np.zeros((W1, WE1), np.int64)
        wseg = np.full((W1, WE1), -1.0, np.float32)
        ft_slot = np.zeros(NI, np.int64)
        wbase = np.full(W1, NI, np.int64)            # pad windows -> zero cols
        for w, (i0, ni) in enumerate(st["w1groups"]):
            e0, e1 = estart[i0], estart[i0 + ni]
            ne = int(e1 - e0)
            assert ne <= WE1 and ni <= WS1
            wsrc[w, :ne] = es[e0:e1]
            wseg[w, :ne] = (ed[e0:e1] - i0).astype(np.float32)
            ft_slot[i0: i0 + ni] = w * WS1 + np.arange(ni)
            wbase[w] = i0
        st["wsrc"], st["wseg"] = wsrc, wseg
        st["ft_slot"] = ft_slot
        st["wbase"] = wbase

    # ---- per-batch gather regions + index/seg arrays ----
    for c in range(NCORES):
        st = cores[c]
        hv2 = np.zeros((B1 * RB, D), BF16)
        g2 = np.zeros((B1, P, RB // 16), np.int16)
        seg = np.full((B1, P, RB // P), -1.0, np.float32)
        for b in range(B1):
            wins = slice(b * WB, (b + 1) * WB)
            src = st["wsrc"][wins].reshape(-1)
            sg = st["wseg"][wins].reshape(-1)
            real = sg >= 0
            uniq = np.unique(src[real])
            if uniq.size == 0:
                uniq = np.array([0], np.int64)
            assert uniq.size <= RB
            hv2[b * RB: b * RB + uniq.size] = h_v_bf[uniq]
            pos = np.zeros(RB, np.int64)
            pos[real] = np.searchsorted(uniq, src[real])
            g2[b] = _wrap_idx16(pos, RB)
            seg[b] = _interleave_f32(sg, RB, -1.0)
        st["hv2"], st["g2"], st["seg"] = hv2, g2, seg

        # window-padded dst table [D, W1*128] (col w*128+s = h_v[islot base+s])
        colitem = np.full(W1 * WS1, -1, np.int64)
        for w, (i0, ni) in enumerate(st["w1groups"]):
            colitem[w * WS1: w * WS1 + ni] = st["it_item"][i0: i0 + ni]
        hvlTw = np.zeros((D, W1 * WS1), BF16)
        cr = colitem >= 0
        hvlTw[:, cr] = h_v_bf[colitem[cr]].T
        st["hvlTw"] = hvlTw

    # ---- stage-2 gather/meta arrays + tables ----
    for c in range(NCORES):
        st = cores[c]
        it_item = st["it_item"]
        real = it_item >= 0
        st["ftg"] = _wrap_idx16(st["ft_slot"], NI)
        st["fexp"] = _wrap_idx16(st["it_tslot"], NI)
        tl = np.zeros((W2, P, TI2), np.float32)
        for w2 in range(W2):
            tl[w2] = _interleave_f32(
                st["it_tgtloc"][w2 * TI2 * P: (w2 + 1) * TI2 * P], TI2 * P,
                -1.0)
        st["tgtloc"] = tl
        hpT = np.zeros((D, NI), BF16)
        hpT[:, real] = h_p_bf[it_item[real]].T
        st["hpT"] = hpT
        htT = np.zeros((D, W2 * WS2), BF16)
        tw = st["twin"].reshape(-1)
        htT[:, tw >= 0] = h_t_bf[tw[tw >= 0]].T
        st["htT"] = htT

    dims = {"NI": NI, "W1": W1, "B1": B1, "W2": W2,
            "NITEM": NITEM, "NTGT": NTGT}
    return dims, cores


# ======================= device program =======================

def build_program(dims):
    import concourse.bacc as bacc
    import concourse.mybir as mybir
    import concourse.tile as tile

    f32 = mybir.dt.float32
    bf16 = mybir.dt.bfloat16
    i16 = mybir.dt.int16
    Alu = mybir.AluOpType
    Act = mybir.ActivationFunctionType
    Ax = mybir.AxisListType

    NI, W1, B1, W2 = (dims[k] for k in ("NI", "W1", "B1", "W2"))
    FTC = W1 * WS1                     # dst-table columns / ft rows
    NW = TI2 * P                       # islots per stage-2 window

    nc = bacc.Bacc("TRN2", target_bir_lowering=False, debug=False,
                   num_devices=NCORES)
    # inputs
    hv2 = nc.dram_tensor("hv2", [B1 * RB, D], bf16, kind="ExternalInput")
    hvlTw = nc.dram_tensor("hvlTw", [D, FTC], bf16, kind="ExternalInput")
    hpT = nc.dram_tensor("hpT", [D, NI], bf16, kind="ExternalInput")
    htT = nc.dram_tensor("htT", [D, W2 * WS2], bf16, kind="ExternalInput")
    qw = nc.dram_tensor("qw", [2 * D, D], f32, kind="ExternalInput")
    rw = nc.dram_tensor("rw", [2 * D, D], f32, kind="ExternalInput")
    pic = nc.dram_tensor("pic", [D, 1], f32, kind="ExternalInput")
    iotab = nc.dram_tensor("iotab", [P, P], bf16, kind="ExternalInput")
    ident = nc.dram_tensor("ident", [P, P], f32, kind="ExternalInput")
    g2d = nc.dram_tensor("g2d", [B1, P, RB // 16], i16, kind="ExternalInput")
    segd = nc.dram_tensor("segd", [B1, P, RB // P], f32, kind="ExternalInput")
    ftgd = nc.dram_tensor("ftgd", [P, NI // 16], i16, kind="ExternalInput")
    fexpd = nc.dram_tensor("fexpd", [P, NI // 16], i16, kind="ExternalInput")
    tgtlocd = nc.dram_tensor("tgtlocd", [W2, P, TI2], f32, kind="ExternalInput")
    # output
    outd = nc.dram_tensor("out", [W2 * WS2, D], f32, kind="ExternalOutput")
    # internal scratch
    hvpiTw = nc.dram_tensor("hvpiTw", [D, FTC], bf16, kind="Internal")
    ftd = nc.dram_tensor("ft", [FTC, D], bf16, kind="Internal")
    fd = nc.dram_tensor("fd", [W2 * WS2, D], bf16, kind="Internal")

    with tile.TileContext(nc) as tc:
        with (
            tc.tile_pool(name="consts", bufs=1) as cp,
            tc.tile_pool(name="weights", bufs=1) as wp,
        ):
            iota_t = cp.tile([P, P], bf16)
            nc.sync.dma_start(out=iota_t[:], in_=iotab[:])
            ident_t = cp.tile([P, P], f32)
            nc.sync.dma_start(out=ident_t[:], in_=ident[:])
            ones_b = cp.tile([P, 1], bf16)
            nc.vector.memset(ones_b[:], 1.0)
            pi_t = cp.tile([D, 1], f32)
            nc.sync.dma_start(out=pi_t[:], in_=pic[:])
            # weights: load f32, cast to bf16 on device
            qwf = wp.tile([P, 2, D], f32)
            nc.sync.dma_start(out=qwf[:, 0, :], in_=qw[0:D, :])
            nc.sync.dma_start(out=qwf[:, 1, :], in_=qw[D: 2 * D, :])
            qwb_t = wp.tile([P, 2, D], bf16)
            nc.scalar.activation(out=qwb_t[:], in_=qwf[:], func=Act.Copy)
            rwf = wp.tile([P, 2, D], f32)
            nc.sync.dma_start(out=rwf[:, 0, :], in_=rw[0:D, :])
            nc.sync.dma_start(out=rwf[:, 1, :], in_=rw[D: 2 * D, :])
            rwb_t = wp.tile([P, 2, D], bf16)
            nc.scalar.activation(out=rwb_t[:], in_=rwf[:], func=Act.Copy)

            # ---- P0: hvpiTw = hvlTw * pi (column blocks) ----
            with tc.tile_pool(name="p0", bufs=3) as p0:
                BLK = 8192
                for o in range(0, FTC, BLK):
                    n = min(BLK, FTC - o)
                    t = p0.tile([D, BLK], bf16, tag="blk")
                    nc.sync.dma_start(out=t[:, :n], in_=hvlTw[:, o:o + n])
                    u = p0.tile([D, BLK], bf16, tag="ublk")
                    nc.vector.tensor_scalar_mul(u[:, :n], t[:, :n], pi_t[:])
                    nc.sync.dma_start(out=hvpiTw[:, o:o + n], in_=u[:, :n])

            # ---- P1: stage-1 batches ----
            with (
                tc.tile_pool(name="idx1", bufs=2) as ip1,
                tc.tile_pool(name="gat", bufs=2) as gp,
                tc.tile_pool(name="ex1", bufs=3) as xp,
                tc.tile_pool(name="mx1", bufs=4) as mp,
                tc.tile_pool(name="sm1", bufs=4) as sm,
                tc.tile_pool(name="fts", bufs=2) as fsp,
                tc.tile_pool(name="psS", bufs=3, space="PSUM") as psS,
                tc.tile_pool(name="psF", bufs=2, space="PSUM") as psF,
            ):
                for b in range(B1):
                    g2t = ip1.tile([P, RB // 16], i16, tag="g2")
                    nc.sync.dma_start(out=g2t[:], in_=g2d[b])
                    segt = ip1.tile([P, RB // P], f32, tag="seg")
                    nc.sync.dma_start(out=segt[:], in_=segd[b])
                    xdw = ip1.tile([P, WB * WS1], bf16, tag="xdw")
                    nc.sync.dma_start(
                        out=xdw[:],
                        in_=hvpiTw[:, b * WB * WS1: (b + 1) * WB * WS1])
                    ftstage = fsp.tile([P, WB, D], bf16, tag="fts")
                    for h in range(2):
                        xsT = gp.tile([P, 1, GH], bf16, tag="xsT")
                        nc.gpsimd.dma_gather(
                            out_ap=xsT[:], in_ap=hv2[b * RB: (b + 1) * RB],
                            idxs_ap=g2t[:, h * GH // 16: (h + 1) * GH // 16],
                            num_idxs=GH, num_idxs_reg=GH, elem_size=D,
                            single_packet=False, transpose=True)
                        xs = gp.tile([P, GH // P, D], bf16, tag="xs")
                        nc.gpsimd.dma_gather(
                            out_ap=xs[:], in_ap=hv2[b * RB: (b + 1) * RB],
                            idxs_ap=g2t[:, h * GH // 16: (h + 1) * GH // 16],
                            num_idxs=GH, num_idxs_reg=GH, elem_size=D,
                            single_packet=False)
                        for wl in range(WB // 2):
                            w = h * (WB // 2) + wl
                            ftp = psF.tile([P, D], f32, space="PSUM", tag="ft")
                            denp = psF.tile([P, 1], f32, space="PSUM",
                                            tag="den")
                            for g in range(2):
                                sp = psS.tile([P, 4 * P], f32, space="PSUM",
                                              tag="sp")
                                for j in range(4):
                                    t = wl * 8 + g * 4 + j   # tile in half
                                    nc.tensor.matmul(
                                        out=sp[:, j * P: (j + 1) * P],
                                        lhsT=xsT[:, 0, t * P: (t + 1) * P],
                                        rhs=xdw[:, w * WS1: (w + 1) * WS1],
                                        start=True, stop=True)
                                ex = xp.tile([P, 4 * P], bf16, tag="ex")
                                nc.scalar.activation(out=ex[:], in_=sp[:],
                                                     func=Act.Exp)
                                for j in range(4):
                                    t = wl * 8 + g * 4 + j
                                    tb = w * 8 + g * 4 + j   # tile in batch
                                    i = g * 4 + j            # tile in window
                                    mx = mp.tile([P, P], bf16, tag="mx")
                                    nc.vector.scalar_tensor_tensor(
                                        out=mx[:], in0=iota_t[:],
                                        scalar=segt[:, tb: tb + 1],
                                        in1=ex[:, j * P: (j + 1) * P],
                                        op0=Alu.is_equal, op1=Alu.mult)
                                    nc.tensor.matmul(
                                        out=ftp[:], lhsT=mx[:],
                                        rhs=xs[:, t, :],
                                        start=(i == 0), stop=(i == 7))
                                    nc.tensor.matmul(
                                        out=denp[:], lhsT=mx[:],
                                        rhs=ones_b[:],
                                        start=(i == 0), stop=(i == 7))
                            denc = sm.tile([P, 1], f32, tag="denc")
                            nc.vector.tensor_scalar_max(denc[:], denp[:],
                                                        1e-30)
                            rec = sm.tile([P, 1], f32, tag="rec")
                            nc.vector.reciprocal(rec[:], denc[:])
                            nc.vector.tensor_scalar_mul(
                                ftstage[:, w, :], ftp[:], rec[:])
                    nc.sync.dma_start(
                        out=ftd[b * WB * WS1: (b + 1) * WB * WS1, :].rearrange(
                            "(w p) d -> p w d", p=P),
                        in_=ftstage[:])

            # ---- P2: stage-2 windows ----
            with (
                tc.tile_pool(name="idx2", bufs=1) as ip2,
                tc.tile_pool(name="big2", bufs=2) as bg,
                tc.tile_pool(name="wk2", bufs=3) as wk2,
                tc.tile_pool(name="ex2", bufs=3) as xp2,
                tc.tile_pool(name="sm2", bufs=4) as sm2,
                tc.tile_pool(name="psA", bufs=1, space="PSUM") as ppA,
                tc.tile_pool(name="psE", bufs=2, space="PSUM") as ppE,
                tc.tile_pool(name="psT", bufs=1, space="PSUM") as ppT,
            ):
                ftgt = ip2.tile([P, NI // 16], i16, tag="ftg")
                nc.sync.dma_start(out=ftgt[:], in_=ftgd[:])
                fext = ip2.tile([P, NI // 16], i16, tag="fex")
                nc.sync.dma_start(out=fext[:], in_=fexpd[:])
                for w2 in range(W2):
                    hpt = bg.tile([P, NW], bf16, tag="hpt")
                    nc.sync.dma_start(out=hpt[:],
                                      in_=hpT[:, w2 * NW: (w2 + 1) * NW])
                    tlt = ip2.tile([P, TI2], f32, tag="tlt")
                    nc.sync.dma_start(out=tlt[:], in_=tgtlocd[w2])
                    ftg = bg.tile([P, TI2, D], bf16, tag="ftgw")
                    ftgT = bg.tile([P, 1, NW], bf16, tag="ftgTw")
                    for o0, n in ((0, 4096), (4096, NW - 4096)):
                        o = w2 * NW + o0
                        nc.gpsimd.dma_gather(
                            out_ap=ftg[:, o0 // P: (o0 + n) // P, :],
                            in_ap=ftd[:],
                            idxs_ap=ftgt[:, o // 16: (o + n) // 16],
                            num_idxs=n, num_idxs_reg=n, elem_size=D,
                            single_packet=False)
                        nc.gpsimd.dma_gather(
                            out_ap=ftgT[:, :, o0: o0 + n],
                            in_ap=ftd[:],
                            idxs_ap=ftgt[:, o // 16: (o + n) // 16],
                            num_idxs=n, num_idxs_reg=n, elem_size=D,
                            single_packet=False, transpose=True)
                    # sweep A: mean + deg
                    meanp = ppA.tile([P, D], f32, space="PSUM", tag="mean")
                    degp = ppA.tile([P, 1], f32, space="PSUM", tag="deg")
                    for i in range(TI2):
                        mask = wk2.tile([P, P], bf16, tag="maskA")
                        nc.vector.tensor_scalar(
                            out=mask[:], in0=iota_t[:],
                            scalar1=tlt[:, i: i + 1], scalar2=None,
                            op0=Alu.is_equal)
                        nc.tensor.matmul(out=meanp[:], lhsT=mask[:],
                                         rhs=ftg[:, i, :],
                                         start=(i == 0), stop=(i == TI2 - 1))
                        nc.tensor.matmul(out=degp[:], lhsT=mask[:],
                                         rhs=ones_b[:],
                                         start=(i == 0), stop=(i == TI2 - 1))
                    degc = sm2.tile([P, 1], f32, tag="degc")
                    nc.vector.tensor_scalar_max(degc[:], degp[:], 1.0)
                    rec2 = sm2.tile([P, 1], f32, tag="rec2")
                    nc.vector.reciprocal(rec2[:], degc[:])
                    mean_sb = wk2.tile([P, D], f32, tag="mean_sb")
                    nc.vector.tensor_scalar_mul(mean_sb[:], meanp[:], rec2[:])
                    trp = ppT.tile([P, P], f32, space="PSUM", tag="trx")
                    nc.tensor.transpose(out=trp[:], in_=mean_sb[:],
                                        identity=ident_t[:])
                    meanT = wk2.tile([P, P], bf16, tag="meanT")
                    nc.scalar.activation(out=meanT[:], in_=trp[:],
                                         func=Act.Copy)
                    htt = wk2.tile([P, P], bf16, tag="htt")
                    nc.sync.dma_start(out=htt[:],
                                      in_=htT[:, w2 * WS2: (w2 + 1) * WS2])
                    fp = ppA.tile([P, D], f32, space="PSUM", tag="fp")
                    nc.tensor.matmul(out=fp[:], lhsT=htt[:], rhs=rwb_t[:, 0, :],
                                     start=True, stop=False)
                    nc.tensor.matmul(out=fp[:], lhsT=meanT[:],
                                     rhs=rwb_t[:, 1, :],
                                     start=False, stop=True)
                    f_sb = wk2.tile([P, D], bf16, tag="f_sb")
                    nc.scalar.activation(out=f_sb[:], in_=fp[:], func=Act.Copy)
                    nc.sync.dma_start(out=fd[w2 * WS2: (w2 + 1) * WS2, :],
                                      in_=f_sb[:])
                    # sweep B
                    fex = bg.tile([P, TI2, D], bf16, tag="fexw")
                    for o0, n in ((0, 4096), (4096, NW - 4096)):
                        o = w2 * NW + o0
                        nc.gpsimd.dma_gather(
                            out_ap=fex[:, o0 // P: (o0 + n) // P, :],
                            in_ap=fd[:],
                            idxs_ap=fext[:, o // 16: (o + n) // 16],
                            num_idxs=n, num_idxs_reg=n, elem_size=D,
                            single_packet=False)
                    outp = ppA.tile([P, D], f32, space="PSUM", tag="outp")
                    for g0 in range(0, TI2, 4):
                        gn = min(4, TI2 - g0)
                        e2p = ppE.tile([P, 4 * P], f32, space="PSUM", tag="e2")
                        for j in range(gn):
                            i = g0 + j
                            nc.tensor.matmul(
                                out=e2p[:, j * P: (j + 1) * P],
                                lhsT=ftgT[:, 0, i * P: (i + 1) * P],
                                rhs=qwb_t[:, 0, :], start=True, stop=False)
                            nc.tensor.matmul(
                                out=e2p[:, j * P: (j + 1) * P],
                                lhsT=hpt[:, i * P: (i + 1) * P],
                                rhs=qwb_t[:, 1, :], start=False, stop=True)
                        e2 = xp2.tile([P, 4 * P], bf16, tag="e2sb")
                        nc.scalar.activation(out=e2[:, : gn * P],
                                             in_=e2p[:, : gn * P],
                                             func=Act.Tanh)
                        scr = xp2.tile([P, 4, P], bf16, tag="scr")
                        nc.vector.tensor_tensor(
                            out=scr[:, :gn, :],
                            in0=e2[:, : gn * P].rearrange(
                                "p (g d) -> p g d", g=gn),
                            in1=fex[:, g0: g0 + gn, :], op=Alu.mult)
                        wc = sm2.tile([P, 4], f32, tag="wc")
                        nc.vector.tensor_reduce(
                            out=wc[:, :gn], in_=scr[:, :gn, :], axis=Ax.X,
                            op=Alu.add)
                        for j in range(gn):
                            i = g0 + j
                            maskw = wk2.tile([P, P], bf16, tag="maskw")
                            nc.vector.tensor_scalar(
                                out=maskw[:], in0=iota_t[:],
                                scalar1=tlt[:, i: i + 1],
                                scalar2=wc[:, j: j + 1],
                                op0=Alu.is_equal, op1=Alu.mult)
                            nc.tensor.matmul(out=outp[:], lhsT=maskw[:],
                                             rhs=ftg[:, i, :],
                                             start=(i == 0),
                                             stop=(i == TI2 - 1))
                    out_sb = wk2.tile([P, D], f32, tag="out_sb")
                    nc.vector.tensor_copy(out=out_sb[:], in_=outp[:])
                    nc.sync.dma_start(out=outd[w2 * WS2: (w2 + 1) * WS2, :],
                                      in_=out_sb[:])
    nc.compile()
    return nc


def make_in_maps(dims, cores, pi_w, q_w, r_w):
    iota_bf = np.tile(np.arange(P, dtype=np.float32), (P, 1)).astype(BF16)
    ident = np.eye(P, dtype=np.float32)
    in_maps = []
    for c in range(NCORES):
        st = cores[c]
        in_maps.append({
            "hv2": st["hv2"],
            "hvlTw": st["hvlTw"],
            "hpT": st["hpT"],
            "htT": st["htT"],
            "qw": np.ascontiguousarray(q_w, np.float32),
            "rw": np.ascontiguousarray(r_w, np.float32),
            "pic": np.ascontiguousarray(pi_w.reshape(D, 1), np.float32),
            "iotab": iota_bf, "ident": ident,
            "g2d": st["g2"], "segd": st["seg"],
            "ftgd": st["ftg"], "fexpd": st["fexp"],
            "tgtlocd": st["tgtloc"],
        })
    return in_maps


def unshard(dims, cores, results):
    NTGT = dims["NTGT"]
    out = np.zeros((NTGT, D), np.float32)
    for c in range(NCORES):
        st = cores[c]
        o = results[c]["out"]
        tw = st["twin"]
        for w2 in range(dims["W2"]):
            sel = tw[w2] >= 0
            out[tw[w2][sel]] = o[w2 * WS2: w2 * WS2 + WS2][sel]
    return out


def kernel(**inputs):
    from concourse.bass_utils import run_bass_kernel_spmd

    h_v = np.asarray(inputs["h_v"], np.float32)
    h_p = np.asarray(inputs["h_p"], np.float32)
    h_t = np.asarray(inputs["h_t"], np.float32)
    pi_w = np.asarray(inputs["pi_w"], np.float32)
    q_w = np.asarray(inputs["q_w"], np.float32)
    r_w = np.asarray(inputs["r_w"], np.float32)
    int_src = np.asarray(inputs["int_src"]).astype(np.int64)
    int_dst = np.asarray(inputs["int_dst"]).astype(np.int64)
    agg_src = np.asarray(inputs["agg_src"]).astype(np.int64)
    agg_dst = np.asarray(inputs["agg_dst"]).astype(np.int64)
    assert np.array_equal(agg_src, np.arange(agg_src.shape[0])), \
        "kernel assumes agg_src == arange (per problem spec fill)"

    dims, cores = preprocess(h_v, h_p, h_t, int_src, int_dst, agg_dst)
    nc = build_program(dims)
    global _LAST_NC
    _LAST_NC = nc
    in_maps = make_in_maps(dims, cores, pi_w, q_w, r_w)
    res = run_bass_kernel_spmd(nc, in_maps, core_ids=list(range(NCORES)))
    return unshard(dims, cores, res.results)


# revision 8
# speedup vs baseline: 2.6465x; 2.6465x over previous
"""Trainium2 Bass kernel for nn_DglAggregator (GNN message passing).

Strategy (8 NeuronCores, SPMD, one uniform program, per-core data):
- Targets are partitioned across cores balanced by stage-1 edge count; each
  core owns its targets' items and ALL stage-1 edges pointing at those items,
  so no cross-core communication is needed.
- Stage 1 (item->item segment softmax + weighted sum): items laid out in
  contiguous "islot" order; windows = consecutive islot ranges (<=128 islots,
  <=1024 edges); 8 windows per batch (8192 edge ranks). Per batch the edge
  source rows are fetched with TWO bf16 dma_gathers from a per-batch
  deduplicated region (<=8192 rows, int16-addressable) of a host-relaid
  table: one normal gather ([edge, d], rhs of the ft matmul) and one
  transpose-mode gather ([d, edge], lhsT of the score matmul).
  Scores: S[e,s] = Xs_e . (h_v[dst_s] * pi) via TensorE against per-window
  dst-slot columns (no per-edge dst gather). Softmax weights: one batched
  exp (Activation engine, 4 tiles per op) + one fused scalar_tensor_tensor
  (mask = (iota==seg) * exp(S)) per tile; ft/den accumulate in PSUM via
  bf16 matmuls. Max-subtraction is skipped (|score| small, exact in f32).
- Stage 2 (item->target): same masked-matmul pattern with bf16 operands;
  per-edge ft rows come from normal + transpose dma_gathers of the stage-1
  output table (the transpose gather replaces PE transpose + copy);
  mask*weight built with fused two-op tensor_scalar.
- Numeric tables (h_v/h_p/h_t) are staged in bfloat16; all arithmetic
  (pi scaling, matmuls, softmax, tanh, means) runs on the NeuronCores with
  f32 PSUM accumulation. Host work is index math, row permutation/layout
  of input tables, and dtype staging.

kernel(**inputs) accepts the FULL unsharded inputs and returns the FULL
[N_TGT, 128] float32 output.
"""
import numpy as np
import ml_dtypes

BF16 = np.dtype(ml_dtypes.bfloat16)

P = 128          # partitions / tile edge
D = 128          # feature dim
NCORES = 8
WE1 = 1024       # stage-1 window edge capacity (8 tiles)
WS1 = 128        # stage-1 window slot capacity
WB = 8           # stage-1 windows per batch
RB = WB * WE1    # edge ranks per batch (8192)
TI2 = 50         # stage-2 tiles per window (6400 item slots)
WS2 = 128        # stage-2 window target capacity
GH = 4096        # gather granularity (half batch)
_LAST_NC = None


def _wrap_idx16(idx: np.ndarray, cap: int) -> np.ndarray:
    """[n<=cap] -> [128, cap/16] int16 (j at [j%16, j//16], replicated x8)."""
    a = np.zeros(cap, np.int64)
    a[: idx.shape[0]] = idx
    assert cap % 16 == 0
    assert a.min() >= 0 and a.max() < 32768, (a.min(), a.max())
    blk = a.reshape(cap // 16, 16).T.astype(np.int16)
    return np.tile(blk, (8, 1))


def _interleave_f32(vals: np.ndarray, cap: int, fill: float) -> np.ndarray:
    """[n] -> [128, cap/128] f32 with value of rank r at [r%128, r//128]."""
    a = np.full(cap, fill, np.float32)
    a[: vals.shape[0]] = vals
    return a.reshape(cap // P, P).T.copy()


def _pack_runs(run_sizes, max_runs, max_total):
    """Greedy pack consecutive runs into groups of whole runs, <=max_runs
    runs and <=max_total total size. Returns list of (start_run, n_runs)."""
    groups = []
    i, n = 0, len(run_sizes)
    while i < n:
        tot, j = 0, i
        while j < n and j - i < max_runs and tot + run_sizes[j] <= max_total:
            tot += run_sizes[j]
            j += 1
        assert j > i, f"run {i} of size {run_sizes[i]} exceeds {max_total}"
        groups.append((i, j - i))
        i = j
    return groups


def preprocess(h_v, h_p, h_t, int_src, int_dst, agg_dst):
    """All graph restructuring. Returns shared dims + per-core arrays."""
    NITEM = h_v.shape[0]
    NTGT = h_t.shape[0]
    int_src = int_src.astype(np.int64)
    int_dst = int_dst.astype(np.int64)
    item_tgt = agg_dst.astype(np.int64)       # item i -> target (agg_src=arange)
    h_v_bf = h_v.astype(BF16)
    h_p_bf = h_p.astype(BF16)
    h_t_bf = h_t.astype(BF16)

    # ---- target -> core, balanced by stage-1 edge load ----
    deg_int = np.bincount(int_dst, minlength=NITEM)
    t_edges = np.bincount(item_tgt, weights=deg_int.astype(np.float64),
                          minlength=NTGT)
    t_items = np.bincount(item_tgt, minlength=NTGT)
    tgt_core = np.zeros(NTGT, np.int64)
    load = np.zeros(NCORES)
    for t in np.argsort(-t_edges, kind="stable"):
        c = int(np.argmin(load))
        tgt_core[t] = c
        load[c] += t_edges[t] + 0.5 * t_items[t]
    item_core = tgt_core[item_tgt]

    cores = []
    for c in range(NCORES):
        tlist = np.where(tgt_core == c)[0]
        items = np.where(item_core == c)[0]
        items = items[np.lexsort((items, item_tgt[items]))]
        cores.append({"targets": tlist, "items": items})

    # ---- stage-2 windows (whole targets, <=WS2 targets, <=TI2*128 islots) ----
    for c in range(NCORES):
        st = cores[c]
        st["w2groups"] = _pack_runs(t_items[st["targets"]], WS2, TI2 * P)
    W2 = max(len(st["w2groups"]) for st in cores)
    NI = W2 * TI2 * P

    for c in range(NCORES):
        st = cores[c]
        tl, items = st["targets"], st["items"]
        it_item = np.full(NI, -1, np.int64)        # islot -> global item
        it_tgtloc = np.full(NI, -1.0, np.float32)  # islot -> window-local tgt
        it_tslot = np.zeros(NI, np.int64)          # islot -> global tgt slot
        twin = np.full((W2, WS2), -1, np.int64)    # window -> global targets
        ipos = 0
        for w2, (t0, ntgt) in enumerate(st["w2groups"]):
            base = w2 * TI2 * P
            off = 0
            for k in range(ntgt):
                t = tl[t0 + k]
                cnt = int(t_items[t])
                sl = slice(base + off, base + off + cnt)
                it_item[sl] = items[ipos: ipos + cnt]
                it_tgtloc[sl] = k
                it_tslot[sl] = w2 * WS2 + k
                twin[w2, k] = t
                ipos += cnt
                off += cnt
        assert ipos == len(items)
        st["it_item"] = it_item
        st["it_tgtloc"] = it_tgtloc
        st["it_tslot"] = it_tslot
        st["twin"] = twin
        islot_of = np.full(NITEM, -1, np.int64)
        real = it_item >= 0
        islot_of[it_item[real]] = np.where(real)[0]
        st["islot_of"] = islot_of

    # ---- stage-1 windows: consecutive islot ranges ----
    for c in range(NCORES):
        st = cores[c]
        emask = item_core[int_dst] == c
        es = int_src[emask]
        ed = st["islot_of"][int_dst[emask]]
        o = np.argsort(ed, kind="stable")
        st["e_src"], st["e_dst"] = es[o], ed[o]
        cnt = np.bincount(st["e_dst"], minlength=NI)
        st["w1groups"] = _pack_runs(cnt, WS1, WE1)   # (islot0, nislots)
        st["islot_cnt"] = cnt
    W1 = max(len(st["w1groups"]) for st in cores)
    W1 = ((W1 + WB - 1) // WB) * WB
    B1 = W1 // WB
    assert W1 * WS1 <= 32768, f"ft table too big for int16: W1={W1}"

    for c in range(NCORES):
        st = cores[c]
        es, ed, cnt = st["e_src"], st["e_dst"], st["islot_cnt"]
        estart = np.concatenate([[0], np.cumsum(cnt)])
        wsrc = np.zeros((W1, WE1), np.int64)
        wseg = np.full((W1, WE1), -1.0, np.float32)
        ft_slot = np.zeros(NI, np.int64)
        wbase = np.full(W1, NI, np.int64)            # pad windows -> zero cols
        for w, (i0, ni) in enumerate(st["w1groups"]):
            e0, e1 = estart[i0], estart[i0 + ni]
            ne = int(e1 - e0)
            assert ne <= WE1 and ni <= WS1
            wsrc[w, :ne] = es[e0:e1]
            wseg[w, :ne] = (ed[e0:e1] - i0).astype(np.float32)
            ft_slot[i0: i0 + ni] = w * WS1 + np.arange(ni)
            wbase[w] = i0
        st["wsrc"], st["wseg"] = wsrc, wseg
        st["ft_slot"] = ft_slot
        st["wbase"] = wbase

    # ---- per-batch gather regions + index/seg arrays ----
    for c in range(NCORES):
        st = cores[c]
        hv2 = np.zeros((B1 * RB, D), BF16)
        g2 = np.zeros((B1, P, RB // 16), np.int16)
        seg = np.full((B1, P, RB // P), -1.0, np.float32)
        for b in range(B1):
            wins = slice(b * WB, (b + 1) * WB)
            src = st["wsrc"][wins].reshape(-1)
            sg = st["wseg"][wins].reshape(-1)
            real = sg >= 0
            uniq = np.unique(src[real])
            if uniq.size == 0:
                uniq = np.array([0], np.int64)
            assert uniq.size <= RB
            hv2[b * RB: b * RB + uniq.size] = h_v_bf[uniq]
            pos = np.zeros(RB, np.int64)
            pos[real] = np.searchsorted(uniq, src[real])
            g2[b] = _wrap_idx16(pos, RB)
            seg[b] = _interleave_f32(sg, RB, -1.0)
        st["hv2"], st["g2"], st["seg"] = hv2, g2, seg

        # window-padded dst table [D, W1*128] (col w*128+s = h_v[islot base+s])
        colitem = np.full(W1 * WS1, -1, np.int64)
        for w, (i0, ni) in enumerate(st["w1groups"]):
            colitem[w * WS1: w * WS1 + ni] = st["it_item"][i0: i0 + ni]
        hvlTw = np.zeros((D, W1 * WS1), BF16)
        cr = colitem >= 0
        hvlTw[:, cr] = h_v_bf[colitem[cr]].T
        st["hvlTw"] = hvlTw

    # ---- stage-2 gather/meta arrays + tables ----
    for c in range(NCORES):
        st = cores[c]
        it_item = st["it_item"]
        real = it_item >= 0
        st["ftg"] = _wrap_idx16(st["ft_slot"], NI)
        st["fexp"] = _wrap_idx16(st["it_tslot"], NI)
        tl = np.zeros((W2, P, TI2), np.float32)
        for w2 in range(W2):
            tl[w2] = _interleave_f32(
                st["it_tgtloc"][w2 * TI2 * P: (w2 + 1) * TI2 * P], TI2 * P,
                -1.0)
        st["tgtloc"] = tl
        hpT = np.zeros((D, NI), BF16)
        hpT[:, real] = h_p_bf[it_item[real]].T
        st["hpT"] = hpT
        htT = np.zeros((D, W2 * WS2), BF16)
        tw = st["twin"].reshape(-1)
        htT[:, tw >= 0] = h_t_bf[tw[tw >= 0]].T
        st["htT"] = htT

    dims = {"NI": NI, "W1": W1, "B1": B1, "W2": W2,
            "NITEM": NITEM, "NTGT": NTGT}
    return dims, cores


# ======================= device program =======================

def build_program(dims):
    import concourse.bacc as bacc
    import concourse.mybir as mybir
    import concourse.tile as tile

    f32 = mybir.dt.float32
    bf16 = mybir.dt.bfloat16
    i16 = mybir.dt.int16
    Alu = mybir.AluOpType
    Act = mybir.ActivationFunctionType
    Ax = mybir.AxisListType

    NI, W1, B1, W2 = (dims[k] for k in ("NI", "W1", "B1", "W2"))
    FTC = W1 * WS1                     # dst-table columns / ft rows
    NW = TI2 * P                       # islots per stage-2 window

    nc = bacc.Bacc("TRN2", target_bir_lowering=False, debug=False,
                   num_devices=NCORES)
    # inputs
    hv2 = nc.dram_tensor("hv2", [B1 * RB, D], bf16, kind="ExternalInput")
    hvlTw = nc.dram_tensor("hvlTw", [D, FTC], bf16, kind="ExternalInput")
    hpT = nc.dram_tensor("hpT", [D, NI], bf16, kind="ExternalInput")
    htT = nc.dram_tensor("htT", [D, W2 * WS2], bf16, kind="ExternalInput")
    qw = nc.dram_tensor("qw", [2 * D, D], f32, kind="ExternalInput")
    rw = nc.dram_tensor("rw", [2 * D, D], f32, kind="ExternalInput")
    pic = nc.dram_tensor("pic", [D, 1], f32, kind="ExternalInput")
    iotab = nc.dram_tensor("iotab", [P, P], bf16, kind="ExternalInput")
    ident = nc.dram_tensor("ident", [P, P], f32, kind="ExternalInput")
    g2d = nc.dram_tensor("g2d", [B1, P, RB // 16], i16, kind="ExternalInput")
    segd = nc.dram_tensor("segd", [B1, P, RB // P], f32, kind="ExternalInput")
    ftgd = nc.dram_tensor("ftgd", [P, NI // 16], i16, kind="ExternalInput")
    fexpd = nc.dram_tensor("fexpd", [P, NI // 16], i16, kind="ExternalInput")
    tgtlocd = nc.dram_tensor("tgtlocd", [W2, P, TI2], f32, kind="ExternalInput")
    # output
    outd = nc.dram_tensor("out", [W2 * WS2, D], f32, kind="ExternalOutput")
    # internal scratch
    ftd = nc.dram_tensor("ft", [FTC, D], bf16, kind="Internal")
    fd = nc.dram_tensor("fd", [W2 * WS2, D], bf16, kind="Internal")

    with tile.TileContext(nc) as tc:
        with (
            tc.tile_pool(name="consts", bufs=1) as cp,
            tc.tile_pool(name="weights", bufs=1) as wp,
        ):
            iota_t = cp.tile([P, P], bf16)
            nc.sync.dma_start(out=iota_t[:], in_=iotab[:])
            ident_t = cp.tile([P, P], f32)
            nc.sync.dma_start(out=ident_t[:], in_=ident[:])
            ident_b = cp.tile([P, P], bf16)
            nc.scalar.activation(out=ident_b[:], in_=ident_t[:], func=Act.Copy)
            ones_b = cp.tile([P, 1], bf16)
            nc.vector.memset(ones_b[:], 1.0)
            pi_t = cp.tile([D, 1], f32)
            nc.sync.dma_start(out=pi_t[:], in_=pic[:])
            # weights: load f32, cast to bf16 on device
            qwf = wp.tile([P, 2, D], f32)
            nc.sync.dma_start(out=qwf[:, 0, :], in_=qw[0:D, :])
            nc.sync.dma_start(out=qwf[:, 1, :], in_=qw[D: 2 * D, :])
            qwb_t = wp.tile([P, 2, D], bf16)
            nc.scalar.activation(out=qwb_t[:], in_=qwf[:], func=Act.Copy)
            rwf = wp.tile([P, 2, D], f32)
            nc.sync.dma_start(out=rwf[:, 0, :], in_=rw[0:D, :])
            nc.sync.dma_start(out=rwf[:, 1, :], in_=rw[D: 2 * D, :])
            rwb_t = wp.tile([P, 2, D], bf16)
            nc.scalar.activation(out=rwb_t[:], in_=rwf[:], func=Act.Copy)

            # ---- P1: stage-1 batches ----
            with (
                tc.tile_pool(name="idx1", bufs=2) as ip1,
                tc.tile_pool(name="gat", bufs=3) as gp,
                tc.tile_pool(name="xsT1", bufs=8) as tp,
                tc.tile_pool(name="ex1", bufs=6) as xp,
                tc.tile_pool(name="mx1", bufs=10) as mp,
                tc.tile_pool(name="sm1", bufs=8) as sm,
                tc.tile_pool(name="fts", bufs=2) as fsp,
                tc.tile_pool(name="psS", bufs=2, space="PSUM") as psS,
                tc.tile_pool(name="psT", bufs=3, space="PSUM") as psT,
                tc.tile_pool(name="psF", bufs=3, space="PSUM") as psF,
            ):
                for b in range(B1):
                    g2t = ip1.tile([P, RB // 16], i16, tag="g2")
                    nc.sync.dma_start(out=g2t[:], in_=g2d[b])
                    segt = ip1.tile([P, RB // P], f32, tag="seg")
                    nc.sync.dma_start(out=segt[:], in_=segd[b])
                    xdw0 = ip1.tile([P, WB * WS1], bf16, tag="xdw0")
                    nc.sync.dma_start(
                        out=xdw0[:],
                        in_=hvlTw[:, b * WB * WS1: (b + 1) * WB * WS1])
                    xdw = ip1.tile([P, WB * WS1], bf16, tag="xdw")
                    nc.vector.tensor_scalar_mul(xdw[:], xdw0[:], pi_t[:])
                    ftstage = fsp.tile([P, WB, D], bf16, tag="fts")
                    for h in range(2):
                        xs = gp.tile([P, GH // P, D], bf16, tag="xs")
                        nc.gpsimd.dma_gather(
                            out_ap=xs[:], in_ap=hv2[b * RB: (b + 1) * RB],
                            idxs_ap=g2t[:, h * GH // 16: (h + 1) * GH // 16],
                            num_idxs=GH, num_idxs_reg=GH, elem_size=D,
                            single_packet=False)
                        for wl in range(WB // 2):
                            w = h * (WB // 2) + wl
                            fdp = psF.tile([P, D + 1], f32, space="PSUM",
                                           tag="ftden")
                            ftp = fdp[:, 0:D]
                            denp = fdp[:, D: D + 1]
                            for g in range(2):
                                # transpose 4 tiles: [e,d] -> [d,e]
                                trp = psT.tile([P, 4 * P], bf16, space="PSUM",
                                               tag="tr")
                                for j in range(4):
                                    t = wl * 8 + g * 4 + j   # tile in half
                                    nc.tensor.transpose(
                                        out=trp[:, j * P: (j + 1) * P],
                                        in_=xs[:, t, :], identity=ident_b[:])
                                xsT = tp.tile([P, 4 * P], bf16, tag="xsT")
                                if (wl * 2 + g) % 2 == 0:
                                    nc.scalar.activation(out=xsT[:],
                                                         in_=trp[:],
                                                         func=Act.Copy)
                                else:
                                    nc.gpsimd.tensor_copy(out=xsT[:],
                                                          in_=trp[:])
                                sp = psS.tile([P, 4 * P], f32, space="PSUM",
                                              tag="sp")
                                for j in range(4):
                                    nc.tensor.matmul(
                                        out=sp[:, j * P: (j + 1) * P],
                                        lhsT=xsT[:, j * P: (j + 1) * P],
                                        rhs=xdw[:, w * WS1: (w + 1) * WS1],
                                        start=True, stop=True)
                                ex = xp.tile([P, 4 * P], bf16, tag="ex")
                                nc.scalar.activation(out=ex[:], in_=sp[:],
                                                     func=Act.Exp)
                                for j in range(4):
                                    t = wl * 8 + g * 4 + j
                                    tb = w * 8 + g * 4 + j   # tile in batch
                                    i = g * 4 + j            # tile in window
                                    mx = mp.tile([P, P], bf16, tag="mx")
                                    nc.vector.scalar_tensor_tensor(
                                        out=mx[:], in0=iota_t[:],
                                        scalar=segt[:, tb: tb + 1],
                                        in1=ex[:, j * P: (j + 1) * P],
                                        op0=Alu.is_equal, op1=Alu.mult)
                                    nc.tensor.matmul(
                                        out=ftp, lhsT=mx[:],
                                        rhs=xs[:, t, :],
                                        start=(i == 0), stop=(i == 7))
                                    nc.tensor.matmul(
                                        out=denp, lhsT=mx[:],
                                        rhs=ones_b[:],
                                        start=(i == 0), stop=(i == 7))
                            denc = sm.tile([P, 1], f32, tag="denc")
                            nc.vector.tensor_scalar_max(denc[:], denp,
                                                        1e-30)
                            rec = sm.tile([P, 1], f32, tag="rec")
                            nc.vector.reciprocal(rec[:], denc[:])
                            nc.vector.tensor_scalar_mul(
                                ftstage[:, w, :], ftp, rec[:])
                    nc.sync.dma_start(
                        out=ftd[b * WB * WS1: (b + 1) * WB * WS1, :].rearrange(
                            "(w p) d -> p w d", p=P),
                        in_=ftstage[:])

            # ---- P2: stage-2 windows ----
            with (
                tc.tile_pool(name="idx2", bufs=1) as ip2,
                tc.tile_pool(name="big2", bufs=2) as bg,
                tc.tile_pool(name="wk2", bufs=3) as wk2,
                tc.tile_pool(name="ex2", bufs=3) as xp2,
                tc.tile_pool(name="sm2", bufs=4) as sm2,
                tc.tile_pool(name="psA", bufs=1, space="PSUM") as ppA,
                tc.tile_pool(name="psE", bufs=2, space="PSUM") as ppE,
                tc.tile_pool(name="psT", bufs=1, space="PSUM") as ppT,
            ):
                ftgt = ip2.tile([P, NI // 16], i16, tag="ftg")
                nc.sync.dma_start(out=ftgt[:], in_=ftgd[:])
                fext = ip2.tile([P, NI // 16], i16, tag="fex")
                nc.sync.dma_start(out=fext[:], in_=fexpd[:])
                for w2 in range(W2):
                    hpt = bg.tile([P, NW], bf16, tag="hpt")
                    nc.sync.dma_start(out=hpt[:],
                                      in_=hpT[:, w2 * NW: (w2 + 1) * NW])
                    tlt = ip2.tile([P, TI2], f32, tag="tlt")
                    nc.sync.dma_start(out=tlt[:], in_=tgtlocd[w2])
                    ftg = bg.tile([P, TI2, D], bf16, tag="ftgw")
                    ftgT = bg.tile([P, 1, NW], bf16, tag="ftgTw")
                    for o0, n in ((0, 4096), (4096, NW - 4096)):
                        o = w2 * NW + o0
                        nc.gpsimd.dma_gather(
                            out_ap=ftg[:, o0 // P: (o0 + n) // P, :],
                            in_ap=ftd[:],
                            idxs_ap=ftgt[:, o // 16: (o + n) // 16],
                            num_idxs=n, num_idxs_reg=n, elem_size=D,
                            single_packet=False)
                        nc.gpsimd.dma_gather(
                            out_ap=ftgT[:, :, o0: o0 + n],
                            in_ap=ftd[:],
                            idxs_ap=ftgt[:, o // 16: (o + n) // 16],
                            num_idxs=n, num_idxs_reg=n, elem_size=D,
                            single_packet=False, transpose=True)
                    # sweep A: mean + deg
                    meanp = ppA.tile([P, D], f32, space="PSUM", tag="mean")
                    degp = ppA.tile([P, 1], f32, space="PSUM", tag="deg")
                    for i in range(TI2):
                        mask = wk2.tile([P, P], bf16, tag="maskA")
                        nc.vector.tensor_scalar(
                            out=mask[:], in0=iota_t[:],
                            scalar1=tlt[:, i: i + 1], scalar2=None,
                            op0=Alu.is_equal)
                        nc.tensor.matmul(out=meanp[:], lhsT=mask[:],
                                         rhs=ftg[:, i, :],
                                         start=(i == 0), stop=(i == TI2 - 1))
                        nc.tensor.matmul(out=degp[:], lhsT=mask[:],
                                         rhs=ones_b[:],
                                         start=(i == 0), stop=(i == TI2 - 1))
                    degc = sm2.tile([P, 1], f32, tag="degc")
                    nc.vector.tensor_scalar_max(degc[:], degp[:], 1.0)
                    rec2 = sm2.tile([P, 1], f32, tag="rec2")
                    nc.vector.reciprocal(rec2[:], degc[:])
                    mean_sb = wk2.tile([P, D], f32, tag="mean_sb")
                    nc.vector.tensor_scalar_mul(mean_sb[:], meanp[:], rec2[:])
                    trp = ppT.tile([P, P], f32, space="PSUM", tag="trx")
                    nc.tensor.transpose(out=trp[:], in_=mean_sb[:],
                                        identity=ident_t[:])
                    meanT = wk2.tile([P, P], bf16, tag="meanT")
                    nc.scalar.activation(out=meanT[:], in_=trp[:],
                                         func=Act.Copy)
                    htt = wk2.tile([P, P], bf16, tag="htt")
                    nc.sync.dma_start(out=htt[:],
                                      in_=htT[:, w2 * WS2: (w2 + 1) * WS2])
                    fp = ppA.tile([P, D], f32, space="PSUM", tag="fp")
                    nc.tensor.matmul(out=fp[:], lhsT=htt[:], rhs=rwb_t[:, 0, :],
                                     start=True, stop=False)
                    nc.tensor.matmul(out=fp[:], lhsT=meanT[:],
                                     rhs=rwb_t[:, 1, :],
                                     start=False, stop=True)
                    f_sb = wk2.tile([P, D], bf16, tag="f_sb")
                    nc.scalar.activation(out=f_sb[:], in_=fp[:], func=Act.Copy)
                    nc.sync.dma_start(out=fd[w2 * WS2: (w2 + 1) * WS2, :],
                                      in_=f_sb[:])
                    # sweep B
                    fex = bg.tile([P, TI2, D], bf16, tag="fexw")
                    for o0, n in ((0, 4096), (4096, NW - 4096)):
                        o = w2 * NW + o0
                        nc.gpsimd.dma_gather(
                            out_ap=fex[:, o0 // P: (o0 + n) // P, :],
                            in_ap=fd[:],
                            idxs_ap=fext[:, o // 16: (o + n) // 16],
                            num_idxs=n, num_idxs_reg=n, elem_size=D,
                            single_packet=False)
                    outp = ppA.tile([P, D], f32, space="PSUM", tag="outp")
                    for g0 in range(0, TI2, 4):
                        gn = min(4, TI2 - g0)
                        e2p = ppE.tile([P, 4 * P], f32, space="PSUM", tag="e2")
                        for j in range(gn):
                            i = g0 + j
                            nc.tensor.matmul(
                                out=e2p[:, j * P: (j + 1) * P],
                                lhsT=ftgT[:, 0, i * P: (i + 1) * P],
                                rhs=qwb_t[:, 0, :], start=True, stop=False)
                            nc.tensor.matmul(
                                out=e2p[:, j * P: (j + 1) * P],
                                lhsT=hpt[:, i * P: (i + 1) * P],
                                rhs=qwb_t[:, 1, :], start=False, stop=True)
                        e2 = xp2.tile([P, 4 * P], bf16, tag="e2sb")
                        nc.scalar.activation(out=e2[:, : gn * P],
                                             in_=e2p[:, : gn * P],
                                             func=Act.Tanh)
                        scr = xp2.tile([P, 4, P], bf16, tag="scr")
                        nc.vector.tensor_tensor(
                            out=scr[:, :gn, :],
                            in0=e2[:, : gn * P].rearrange(
                                "p (g d) -> p g d", g=gn),
                            in1=fex[:, g0: g0 + gn, :], op=Alu.mult)
                        wc = sm2.tile([P, 4], f32, tag="wc")
                        nc.vector.tensor_reduce(
                            out=wc[:, :gn], in_=scr[:, :gn, :], axis=Ax.X,
                            op=Alu.add)
                        for j in range(gn):
                            i = g0 + j
                            maskw = wk2.tile([P, P], bf16, tag="maskw")
                            nc.vector.tensor_scalar(
                                out=maskw[:], in0=iota_t[:],
                                scalar1=tlt[:, i: i + 1],
                                scalar2=wc[:, j: j + 1],
                                op0=Alu.is_equal, op1=Alu.mult)
                            nc.tensor.matmul(out=outp[:], lhsT=maskw[:],
                                             rhs=ftg[:, i, :],
                                             start=(i == 0),
                                             stop=(i == TI2 - 1))
                    out_sb = wk2.tile([P, D], f32, tag="out_sb")
                    nc.vector.tensor_copy(out=out_sb[:], in_=outp[:])
                    nc.sync.dma_start(out=outd[w2 * WS2: (w2 + 1) * WS2, :],
                                      in_=out_sb[:])
    nc.compile()
    return nc


def make_in_maps(dims, cores, pi_w, q_w, r_w):
    iota_bf = np.tile(np.arange(P, dtype=np.float32), (P, 1)).astype(BF16)
    ident = np.eye(P, dtype=np.float32)
    in_maps = []
    for c in range(NCORES):
        st = cores[c]
        in_maps.append({
            "hv2": st["hv2"],
            "hvlTw": st["hvlTw"],
            "hpT": st["hpT"],
            "htT": st["htT"],
            "qw": np.ascontiguousarray(q_w, np.float32),
            "rw": np.ascontiguousarray(r_w, np.float32),
            "pic": np.ascontiguousarray(pi_w.reshape(D, 1), np.float32),
            "iotab": iota_bf, "ident": ident,
            "g2d": st["g2"], "segd": st["seg"],
            "ftgd": st["ftg"], "fexpd": st["fexp"],
            "tgtlocd": st["tgtloc"],
        })
    return in_maps


def unshard(dims, cores, results):
    NTGT = dims["NTGT"]
    out = np.zeros((NTGT, D), np.float32)
    for c in range(NCORES):
        st = cores[c]
        o = results[c]["out"]
        tw = st["twin"]
        for w2 in range(dims["W2"]):
            sel = tw[w2] >= 0
            out[tw[w2][sel]] = o[w2 * WS2: w2 * WS2 + WS2][sel]
    return out


def kernel(**inputs):
    from concourse.bass_utils import run_bass_kernel_spmd

    h_v = np.asarray(inputs["h_v"], np.float32)
    h_p = np.asarray(inputs["h_p"], np.float32)
    h_t = np.asarray(inputs["h_t"], np.float32)
    pi_w = np.asarray(inputs["pi_w"], np.float32)
    q_w = np.asarray(inputs["q_w"], np.float32)
    r_w = np.asarray(inputs["r_w"], np.float32)
    int_src = np.asarray(inputs["int_src"]).astype(np.int64)
    int_dst = np.asarray(inputs["int_dst"]).astype(np.int64)
    agg_src = np.asarray(inputs["agg_src"]).astype(np.int64)
    agg_dst = np.asarray(inputs["agg_dst"]).astype(np.int64)
    assert np.array_equal(agg_src, np.arange(agg_src.shape[0])), \
        "kernel assumes agg_src == arange (per problem spec fill)"

    dims, cores = preprocess(h_v, h_p, h_t, int_src, int_dst, agg_dst)
    nc = build_program(dims)
    global _LAST_NC
    _LAST_NC = nc
    in_maps = make_in_maps(dims, cores, pi_w, q_w, r_w)
    res = run_bass_kernel_spmd(nc, in_maps, core_ids=list(range(NCORES)))
    return unshard(dims, cores, res.results)
